# revision 6
# baseline (speedup 1.0000x reference)
"""Trainium2 Bass kernel for nn_CrossAttention (sparse_attention).

Strategy: data-parallel over batch N=8 across the 8 NeuronCores; each core
computes one batch element's full attention independently (no collectives).

Per-core math (per head h, S=1024, D=64):
  vals/keys/qrs = per-head linear projections (bias folded in via an
    augmented ones-row; the 1/sqrt(EMB) softmax scale folded into Wq).
  scoresT[j,i] = keys[j]. qrs[i]          (computed transposed, strict j>i)
  P = exp(scoresT) with masked entries exactly 0 (multiply by 0/1 mask on
    diagonal blocks; lower blocks never computed). Softmax denominators come
    free from a ones-column appended to V in the AV matmul; the fully-masked
    last row (uniform 1/S) is reproduced exactly by a rank-1 constant matmul.
  Sbias[i,j] = QE-skew = vals[i] . E[S-1-i+j]  for j<=i (full lower tri).
    Implemented with ZERO reversals/transposes of B:
      B_rev[i-tile, c] = vals[i] . E[c + S-128-q0]   (plain slice of E^T)
      Sbias[i,j] = B_rev[i, 127 - (i%128) + j]       (diagonal SBUF AP)
    then XBAR dma_start_transpose per 128-block -> SbiasT for the matmul.
  z = (exp @ V)/Z + SbiasT-matmul, combined in one scalar_tensor_tensor.
  zT via PE transpose; out = Wo @ zT + bo (per 128-chunk), output transposed.
"""
import os

os.environ.setdefault("MYCRO_LOCAL_CACHE", "1")

import numpy as np
import ml_dtypes

import concourse.bass as bass
import concourse.mybir as mybir
import concourse.tile as tile
from concourse import bacc
from concourse.bass_utils import run_bass_kernel_spmd
from concourse.masks import make_identity

N, S, EMB, H, D = 8, 1024, 512, 8, 64
DA = D + 1              # augmented (ones row)
NCORES = 8
BF = mybir.dt.bfloat16
F32 = mybir.dt.float32
bf16 = ml_dtypes.bfloat16
QT = S // 128           # 8 q/l tiles per head


def build_nc(heads=H):
    nc = bacc.Bacc("TRN2", target_bir_lowering=False, debug=False,
                   num_devices=NCORES)

    # ---- DRAM I/O ----
    # augmented transposed inputs per head: [H, DA, S]
    vTa = nc.dram_tensor("vTa", [H, DA, S], BF, kind="ExternalInput")
    kTa = nc.dram_tensor("kTa", [H, DA, S], BF, kind="ExternalInput")
    qTa = nc.dram_tensor("qTa", [H, DA, S], BF, kind="ExternalInput")
    WvTa = nc.dram_tensor("WvTa", [DA, D], BF, kind="ExternalInput")
    WkTa = nc.dram_tensor("WkTa", [DA, D], BF, kind="ExternalInput")
    WqTa = nc.dram_tensor("WqTa", [DA, D], BF, kind="ExternalInput")
    WvN = nc.dram_tensor("WvN", [DA, DA], BF, kind="ExternalInput")
    ETa = nc.dram_tensor("ETa", [D, S], BF, kind="ExternalInput")
    WoT = nc.dram_tensor("WoT", [EMB, EMB], BF, kind="ExternalInput")
    boC = nc.dram_tensor("boC", [4, 128, 1], F32, kind="ExternalInput")
    M01 = nc.dram_tensor("M01", [128, 128], BF, kind="ExternalInput")
    E127 = nc.dram_tensor("E127", [128, 128], BF, kind="ExternalInput")
    outT = nc.dram_tensor("outT", [EMB, S], F32, kind="ExternalOutput")

    from contextlib import ExitStack
    with tile.TileContext(nc) as tc, ExitStack() as ctx:
        consts = ctx.enter_context(tc.tile_pool(name="consts", bufs=1))
        inp = ctx.enter_context(tc.tile_pool(name="inp", bufs=2))
        proj = ctx.enter_context(tc.tile_pool(name="proj", bufs=2))
        attn = ctx.enter_context(tc.tile_pool(name="attn", bufs=2))
        small = ctx.enter_context(tc.tile_pool(name="small", bufs=4))
        zt_pool = ctx.enter_context(tc.tile_pool(name="ztp", bufs=1))
        outp = ctx.enter_context(tc.tile_pool(name="outp", bufs=2))
        # PSUM pools (8 banks total): big rotating pool + small dedicated
        ps_big = ctx.enter_context(tc.tile_pool(name="ps_big", bufs=4,
                                                space="PSUM"))
        ps_zp = ctx.enter_context(tc.tile_pool(name="ps_zp", bufs=1,
                                               space="PSUM"))
        ps_zs = ctx.enter_context(tc.tile_pool(name="ps_zs", bufs=1,
                                               space="PSUM"))
        ps_zt = ctx.enter_context(tc.tile_pool(name="ps_zt", bufs=2,
                                               space="PSUM"))
        dram = ctx.enter_context(tc.tile_pool(name="dram", bufs=2,
                                              space="DRAM"))
        if True:
            # ---- load constants ----
            c_wv = consts.tile([DA, D], BF)
            c_wk = consts.tile([DA, D], BF)
            c_wq = consts.tile([DA, D], BF)
            c_wvn = consts.tile([DA, DA], BF)
            c_et = consts.tile([D, S], BF)
            c_wo = consts.tile([128, 4, EMB], BF)   # [e-part, e-chunk, e_out]
            c_bo = consts.tile([128, 4, 1], F32)
            c_m01 = consts.tile([128, 128], BF)
            c_e127 = consts.tile([128, 128], BF)
            c_id = consts.tile([128, 128], BF)
            nc.sync.dma_start(out=c_wv[:], in_=WvTa[:])
            nc.sync.dma_start(out=c_wk[:], in_=WkTa[:])
            nc.sync.dma_start(out=c_wq[:], in_=WqTa[:])
            nc.sync.dma_start(out=c_wvn[:], in_=WvN[:])
            nc.sync.dma_start(out=c_et[:], in_=ETa[:])
            nc.sync.dma_start(
                out=c_wo[:], in_=WoT[:].rearrange("(c p) e -> p c e", p=128))
            nc.sync.dma_start(
                out=c_bo[:], in_=boC[:].rearrange("c p one -> p c one"))
            nc.sync.dma_start(out=c_m01[:], in_=M01[:])
            nc.sync.dma_start(out=c_e127[:], in_=E127[:])
            make_identity(nc, c_id[:])

            # zT chunks: [128 (= 2 heads of d), S] bf16, 4 of them
            zTc = [zt_pool.tile([128, S], BF, tag=f"ztc{i}", name=f"ztc{i}")
                   for i in range(4)]

            for h in range(heads):
                # ---- stage inputs ----
                xv = inp.tile([DA, S], BF, tag="xv")
                xk = inp.tile([DA, S], BF, tag="xk")
                xq = inp.tile([DA, S], BF, tag="xq")
                nc.sync.dma_start(out=xv[:], in_=vTa[h])
                nc.sync.dma_start(out=xk[:], in_=kTa[h])
                nc.sync.dma_start(out=xq[:], in_=qTa[h])

                # ---- transposed projections kT/qT/vT [64, S] ----
                kT = proj.tile([D, S], BF, tag="kT")
                qT = proj.tile([D, S], BF, tag="qT")
                vT = proj.tile([D, S], BF, tag="vT")
                for (dst, w, x) in ((kT, c_wk, xk), (qT, c_wq, xq),
                                    (vT, c_wv, xv)):
                    for sl in range(2):
                        cols = bass.ts(sl, 512)
                        pm = ps_big.tile([128, 512], F32, tag="mm")
                        nc.tensor.matmul(pm[:D, :], w[:], x[:, cols],
                                         start=True, stop=True)
                        nc.scalar.copy(out=dst[:, cols], in_=pm[:D, :])

                # ---- normal-form vals with ones column: vN [128, QT, DA] ----
                vN = proj.tile([128, QT, DA], BF, tag="vN")
                for g in range(2):  # two psum groups of 4 l-tiles
                    pm = ps_big.tile([128, 4 * DA], F32, tag="mm")
                    for lt4 in range(4):
                        lt = g * 4 + lt4
                        nc.tensor.matmul(
                            pm[:, lt4 * DA:(lt4 + 1) * DA],
                            xv[:, bass.ts(lt, 128)], c_wvn[:],
                            start=True, stop=True)
                    nc.vector.tensor_copy(
                        out=vN[:, g * 4:(g + 1) * 4, :].rearrange(
                            "p a b -> p (a b)"),
                        in_=pm[:])

                # ---- scoresT -> exp (per l-tile lb), strict upper j>i ----
                # expT[lb]: [128, 128*(lb+1)] bf16
                expT = []
                for lb in range(QT):
                    wl = 128 * (lb + 1)
                    et = attn.tile([128, wl], BF, tag=f"expT{lb}")
                    for c0 in range(0, wl, 512):
                        cw = min(512, wl - c0)
                        pm = ps_big.tile([128, 512], F32, tag="mm")
                        nc.tensor.matmul(pm[:, :cw],
                                         kT[:, bass.ts(lb, 128)],
                                         qT[:, c0:c0 + cw],
                                         start=True, stop=True)
                        nc.scalar.activation(
                            out=et[:, c0:c0 + cw], in_=pm[:, :cw],
                            func=mybir.ActivationFunctionType.Exp)
                    # diagonal block: keep only j>i  (p>f within block)
                    nc.vector.tensor_mul(et[:, bass.ts(lb, 128)],
                                         et[:, bass.ts(lb, 128)], c_m01[:])
                    expT.append(et)

                # ---- B_rev + skew-transpose -> SbT blocks ----
                # B_rev[qi]: [128, 128*(qi+1) + 128] bf16 (tail 128 zeroed)
                sbT = []
                for qi in range(QT):
                    w = 128 * (qi + 1)
                    wp = w + 128
                    br = attn.tile([128, wp], BF, tag=f"brev{qi}")
                    nc.vector.memset(br[:, w:wp], 0.0)
                    for c0 in range(0, w, 512):
                        cw = min(512, w - c0)
                        pm = ps_big.tile([128, 512], F32, tag="mm")
                        nc.tensor.matmul(
                            pm[:, :cw],
                            vT[:, bass.ts(qi, 128)],
                            c_et[:, S - 128 - qi * 128 + c0:
                                 S - 128 - qi * 128 + c0 + cw],
                            start=True, stop=True)
                        nc.vector.tensor_copy(out=br[:, c0:c0 + cw],
                                              in_=pm[:, :cw])
                    # walrus rejects diagonal SBUF APs (partition-step
                    # semantics) — bounce through DRAM, where APs are linear.
                    brd = dram.tile([128, wp], BF, tag=f"brd{qi}",
                                    name=f"brd{h}_{qi}")
                    nc.sync.dma_start(out=brd[:], in_=br[:])
                    brdap = brd[:]
                    row = []
                    for jb in range(qi + 1):
                        st = attn.tile([128, 128], BF, tag=f"sbT{qi}_{jb}")
                        diag = bass.AP(
                            tensor=brdap.tensor,
                            offset=brdap.offset + 127 + jb * 128,
                            ap=[[wp - 1, 128], [1, 128]])
                        nc.sync.dma_start_transpose(out=st[:], in_=diag)
                        row.append(st)
                    sbT.append(row)

                # ---- AV + softmax-normalize + combine ----
                for qi in range(QT):
                    zp = ps_zp.tile([128, DA], F32, tag="zp")
                    n_mm = (QT - qi) + (QT if qi == QT - 1 else 0)
                    mi = 0
                    for lb in range(qi, QT):
                        nc.tensor.matmul(
                            zp[:], expT[lb][:, bass.ts(qi, 128)],
                            vN[:, lb, :],
                            start=(mi == 0), stop=(mi == n_mm - 1))
                        mi += 1
                    if qi == QT - 1:
                        for lb in range(QT):  # uniform last row fix
                            nc.tensor.matmul(
                                zp[:], c_e127[:], vN[:, lb, :],
                                start=(mi == 0), stop=(mi == n_mm - 1))
                            mi += 1
                    zs = ps_zs.tile([128, D], F32, tag="zs")
                    for jb in range(qi + 1):
                        nc.tensor.matmul(
                            zs[:], sbT[qi][jb][:], vN[:, jb, :D],
                            start=(jb == 0), stop=(jb == qi))
                    rcp = small.tile([128, 1], F32, tag="rcp")
                    nc.vector.reciprocal(rcp[:], zp[:, D:DA])
                    # DVE has a single PSUM port: zp and zs cannot be read in
                    # one op, so normalize then add in two steps.
                    zpn = small.tile([128, D], F32, tag="zpn")
                    nc.vector.tensor_scalar_mul(zpn[:], zp[:, :D], rcp[:])
                    zrow = small.tile([128, D], BF, tag="zrow")
                    nc.vector.tensor_add(zrow[:], zpn[:], zs[:])
                    # transpose z [128, 64] -> [64, 128] and place into zTc
                    zt = ps_zt.tile([D, 128], BF, tag="zt")
                    nc.tensor.transpose(zt[:], zrow[:], c_id[:])
                    nc.scalar.copy(
                        out=zTc[h // 2][(h % 2) * D:(h % 2) * D + D,
                                        bass.ts(qi, 128)],
                        in_=zt[:])

            # ---- output projection: outT[m-chunk] = WoT-chunks @ zTc + bo ----
            for m in range(4):
                for sl in range(2):
                    pm = ps_big.tile([128, 512], F32, tag="mm")
                    for kc in range(4):
                        nc.tensor.matmul(
                            pm[:], c_wo[:, kc, bass.ts(m, 128)],
                            zTc[kc][:, bass.ts(sl, 512)],
                            start=(kc == 0), stop=(kc == 3))
                    ot = outp.tile([128, 512], F32, tag="ot")
                    nc.scalar.add(out=ot[:], in_=pm[:], add=c_bo[:, m, :])
                    nc.sync.dma_start(
                        out=outT[bass.ts(m, 128), bass.ts(sl, 512)],
                        in_=ot[:])

    nc.compile()
    return nc


_nc_cache = {}


def _get_nc():
    if "nc" not in _nc_cache:
        _nc_cache["nc"] = build_nc()
    return _nc_cache["nc"]


def _prep_inputs(v, k, q, Wv, bv, Wk, bk, Wq, bq, E, Wo, bo):
    """Host-side layout prep (numpy only). Returns per-core input maps."""
    f = np.asarray
    v, k, q = f(v, np.float32), f(k, np.float32), f(q, np.float32)
    Wv, bv = f(Wv, np.float32), f(bv, np.float32)
    Wk, bk = f(Wk, np.float32), f(bk, np.float32)
    Wq, bq = f(Wq, np.float32), f(bq, np.float32)
    E, Wo, bo = f(E, np.float32), f(Wo, np.float32), f(bo, np.float32)

    scale = 1.0 / np.sqrt(np.float32(EMB))

    def prep_x(x):  # (N,S,EMB) -> (N,H,DA,S) bf16
        xt = x.reshape(N, S, H, D).transpose(0, 2, 3, 1)
        ones = np.ones((N, H, 1, S), np.float32)
        return np.ascontiguousarray(
            np.concatenate([xt, ones], axis=2)).astype(bf16)

    vT_, kT_, qT_ = prep_x(v), prep_x(k), prep_x(q)

    def prep_w(W_, b_, s=1.0):
        return np.concatenate(
            [W_.T * s, b_[None, :] * s], 0).astype(bf16)

    wv, wk, wq = prep_w(Wv, bv), prep_w(Wk, bk), prep_w(Wq, bq, scale)
    wvn = np.zeros((DA, DA), np.float32)
    wvn[:D, :D] = Wv.T
    wvn[D, :D] = bv
    wvn[D, D] = 1.0
    wvn = wvn.astype(bf16)
    eta = np.ascontiguousarray(E[0].T).astype(bf16)          # [D, S]
    wot = np.ascontiguousarray(Wo.T).astype(bf16)            # [e, e_out]
    boc = np.ascontiguousarray(bo.reshape(4, 128, 1)).astype(np.float32)
    m01 = np.tril(np.ones((128, 128), np.float32), -1).astype(bf16)
    e127 = np.zeros((128, 128), np.float32)
    e127[:, 127] = 1.0 / S
    e127 = e127.astype(bf16)

    shared = {"WvTa": wv, "WkTa": wk, "WqTa": wq, "WvN": wvn, "ETa": eta,
              "WoT": wot, "boC": boc, "M01": m01, "E127": e127}
    return [
        {"vTa": np.ascontiguousarray(vT_[n]),
         "kTa": np.ascontiguousarray(kT_[n]),
         "qTa": np.ascontiguousarray(qT_[n]), **shared}
        for n in range(N)
    ]


def kernel(v, k, q, Wv, bv, Wk, bk, Wq, bq, E, Wo, bo):
    in_maps = _prep_inputs(v, k, q, Wv, bv, Wk, bk, Wq, bq, E, Wo, bo)
    nc = _get_nc()
    res = run_bass_kernel_spmd(nc, in_maps, list(range(NCORES)))
    out = np.stack([res.results[n]["outT"] for n in range(N)])  # (N,EMB,S)
    return np.ascontiguousarray(out.transpose(0, 2, 1))


# revision 8
# speedup vs baseline: 1.5020x; 1.5020x over previous
"""Trainium2 Bass kernel for nn_CrossAttention (sparse_attention).

Strategy: data-parallel over batch N=8 across the 8 NeuronCores; each core
computes one batch element's full attention independently (no collectives).

Per-core math (per head h, S=1024, D=64):
  vals/keys/qrs = per-head linear projections (bias folded in via an
    augmented ones-row; the 1/sqrt(EMB) softmax scale folded into Wq).
  scoresT[j,i] = keys[j]. qrs[i]          (computed transposed, strict j>i)
  P = exp(scoresT) with masked entries exactly 0 (multiply by 0/1 mask on
    diagonal blocks; lower blocks never computed). Softmax denominators come
    free from a ones-column appended to V in the AV matmul; the fully-masked
    last row (uniform 1/S) is reproduced exactly by a rank-1 constant matmul.
  Sbias[i,j] = QE-skew = vals[i] . E[S-1-i+j]  for j<=i (full lower tri).
    Implemented with ZERO reversals/transposes of B:
      B_rev[i-tile, c] = vals[i] . E[c + S-128-q0]   (plain slice of E^T)
      Sbias[i,j] = B_rev[i, 127 - (i%128) + j]       (diagonal SBUF AP)
    then XBAR dma_start_transpose per 128-block -> SbiasT for the matmul.
  z = (exp @ V)/Z + SbiasT-matmul, combined in one scalar_tensor_tensor.
  zT via PE transpose; out = Wo @ zT + bo (per 128-chunk), output transposed.
"""
import os

os.environ.setdefault("MYCRO_LOCAL_CACHE", "1")

import numpy as np
import ml_dtypes

import concourse.bass as bass
import concourse.mybir as mybir
import concourse.tile as tile
from concourse import bacc
from concourse.bass_utils import run_bass_kernel_spmd
from concourse.masks import make_identity

N, S, EMB, H, D = 8, 1024, 512, 8, 64
DA = D + 1              # augmented (ones row)
NCORES = 8
BF = mybir.dt.bfloat16
F32 = mybir.dt.float32
bf16 = ml_dtypes.bfloat16
QT = S // 128           # 8 q/l tiles per head


def build_nc(heads=H):
    nc = bacc.Bacc("TRN2", target_bir_lowering=False, debug=False,
                   num_devices=NCORES)

    # ---- DRAM I/O ----
    # augmented transposed inputs per head: [H, DA, S]
    vTa = nc.dram_tensor("vTa", [H, DA, S], BF, kind="ExternalInput")
    kTa = nc.dram_tensor("kTa", [H, DA, S], BF, kind="ExternalInput")
    qTa = nc.dram_tensor("qTa", [H, DA, S], BF, kind="ExternalInput")
    WvTa = nc.dram_tensor("WvTa", [DA, D], BF, kind="ExternalInput")
    WkTa = nc.dram_tensor("WkTa", [DA, D], BF, kind="ExternalInput")
    WqTa = nc.dram_tensor("WqTa", [DA, D], BF, kind="ExternalInput")
    WvN = nc.dram_tensor("WvN", [DA, DA], BF, kind="ExternalInput")
    ETa = nc.dram_tensor("ETa", [D, S], BF, kind="ExternalInput")
    WoT = nc.dram_tensor("WoT", [EMB, EMB], BF, kind="ExternalInput")
    boC = nc.dram_tensor("boC", [4, 128, 1], F32, kind="ExternalInput")
    M01 = nc.dram_tensor("M01", [128, 128], BF, kind="ExternalInput")
    E127 = nc.dram_tensor("E127", [128, 128], BF, kind="ExternalInput")
    outT = nc.dram_tensor("outT", [EMB, S], F32, kind="ExternalOutput")

    from contextlib import ExitStack
    with tile.TileContext(nc) as tc, ExitStack() as ctx:
        consts = ctx.enter_context(tc.tile_pool(name="consts", bufs=1))
        inp = ctx.enter_context(tc.tile_pool(name="inp", bufs=2))
        proj = ctx.enter_context(tc.tile_pool(name="proj", bufs=2))
        attn = ctx.enter_context(tc.tile_pool(name="attn", bufs=2))
        small = ctx.enter_context(tc.tile_pool(name="small", bufs=4))
        zt_pool = ctx.enter_context(tc.tile_pool(name="ztp", bufs=1))
        outp = ctx.enter_context(tc.tile_pool(name="outp", bufs=2))
        # PSUM pools (8 banks total): big rotating pool + small dedicated
        ps_big = ctx.enter_context(tc.tile_pool(name="ps_big", bufs=4,
                                                space="PSUM"))
        ps_zp = ctx.enter_context(tc.tile_pool(name="ps_zp", bufs=1,
                                               space="PSUM"))
        ps_zs = ctx.enter_context(tc.tile_pool(name="ps_zs", bufs=1,
                                               space="PSUM"))
        ps_zt = ctx.enter_context(tc.tile_pool(name="ps_zt", bufs=2,
                                               space="PSUM"))
        dram = ctx.enter_context(tc.tile_pool(name="dram", bufs=2,
                                              space="DRAM"))
        if True:
            # ---- load constants ----
            c_wv = consts.tile([DA, D], BF)
            c_wk = consts.tile([DA, D], BF)
            c_wq = consts.tile([DA, D], BF)
            c_wvn = consts.tile([DA, DA], BF)
            c_et = consts.tile([D, S], BF)
            c_wo = consts.tile([128, 4, EMB], BF)   # [e-part, e-chunk, e_out]
            c_bo = consts.tile([128, 4, 1], F32)
            c_m01 = consts.tile([128, 128], BF)
            c_e127 = consts.tile([128, 128], BF)
            c_id = consts.tile([128, 128], BF)
            nc.sync.dma_start(out=c_wv[:], in_=WvTa[:])
            nc.sync.dma_start(out=c_wk[:], in_=WkTa[:])
            nc.sync.dma_start(out=c_wq[:], in_=WqTa[:])
            nc.sync.dma_start(out=c_wvn[:], in_=WvN[:])
            nc.sync.dma_start(out=c_et[:], in_=ETa[:])
            nc.sync.dma_start(
                out=c_wo[:], in_=WoT[:].rearrange("(c p) e -> p c e", p=128))
            nc.sync.dma_start(
                out=c_bo[:], in_=boC[:].rearrange("c p one -> p c one"))
            nc.sync.dma_start(out=c_m01[:], in_=M01[:])
            nc.sync.dma_start(out=c_e127[:], in_=E127[:])
            make_identity(nc, c_id[:])

            # zT chunks: [128 (= 2 heads of d), S] bf16, 4 of them
            zTc = [zt_pool.tile([128, S], BF, tag=f"ztc{i}", name=f"ztc{i}")
                   for i in range(4)]

            for h in range(heads):
                # ---- stage inputs ----
                xv = inp.tile([DA, S], BF, tag="xv")
                xk = inp.tile([DA, S], BF, tag="xk")
                xq = inp.tile([DA, S], BF, tag="xq")
                nc.gpsimd.dma_start(out=xv[:], in_=vTa[h])
                nc.gpsimd.dma_start(out=xk[:], in_=kTa[h])
                nc.gpsimd.dma_start(out=xq[:], in_=qTa[h])

                # ---- transposed projections kT/qT/vT [64, S] ----
                kT = proj.tile([D, S], BF, tag="kT")
                qT = proj.tile([D, S], BF, tag="qT")
                vT = proj.tile([D, S], BF, tag="vT")
                for (dst, w, x) in ((kT, c_wk, xk), (qT, c_wq, xq),
                                    (vT, c_wv, xv)):
                    for sl in range(2):
                        cols = bass.ts(sl, 512)
                        pm = ps_big.tile([128, 512], F32, tag="mm")
                        nc.tensor.matmul(pm[:D, :], w[:], x[:, cols],
                                         start=True, stop=True)
                        nc.scalar.copy(out=dst[:, cols], in_=pm[:D, :])

                # ---- normal-form vals with ones column: vN [128, QT, DA] ----
                vN = proj.tile([128, QT, DA], BF, tag="vN")
                for g in range(2):  # two psum groups of 4 l-tiles
                    pm = ps_big.tile([128, 4 * DA], F32, tag="mm")
                    for lt4 in range(4):
                        lt = g * 4 + lt4
                        nc.tensor.matmul(
                            pm[:, lt4 * DA:(lt4 + 1) * DA],
                            xv[:, bass.ts(lt, 128)], c_wvn[:],
                            start=True, stop=True)
                    nc.vector.tensor_copy(
                        out=vN[:, g * 4:(g + 1) * 4, :].rearrange(
                            "p a b -> p (a b)"),
                        in_=pm[:])

                # ---- scoresT -> exp (per l-tile lb), strict upper j>i ----
                # expT[lb]: [128, 128*(lb+1)] bf16
                expT = []
                for lb in range(QT):
                    wl = 128 * (lb + 1)
                    et = attn.tile([128, wl], BF, tag=f"expT{lb}")
                    for c0 in range(0, wl, 512):
                        cw = min(512, wl - c0)
                        pm = ps_big.tile([128, 512], F32, tag="mm")
                        nc.tensor.matmul(pm[:, :cw],
                                         kT[:, bass.ts(lb, 128)],
                                         qT[:, c0:c0 + cw],
                                         start=True, stop=True)
                        nc.scalar.activation(
                            out=et[:, c0:c0 + cw], in_=pm[:, :cw],
                            func=mybir.ActivationFunctionType.Exp)
                    # diagonal block: keep only j>i  (p>f within block)
                    nc.vector.tensor_mul(et[:, bass.ts(lb, 128)],
                                         et[:, bass.ts(lb, 128)], c_m01[:])
                    expT.append(et)

                # ---- B_rev + skew-transpose -> SbT blocks ----
                # B_rev[qi]: [128, 128*(qi+1) + 128] bf16 (tail 128 zeroed)
                sbT = []
                for qi in range(QT):
                    w = 128 * (qi + 1)
                    wp = w + 128
                    br = attn.tile([128, wp], BF, tag=f"brev{qi}")
                    nc.vector.memset(br[:, w:wp], 0.0)
                    for c0 in range(0, w, 512):
                        cw = min(512, w - c0)
                        pm = ps_big.tile([128, 512], F32, tag="mm")
                        nc.tensor.matmul(
                            pm[:, :cw],
                            vT[:, bass.ts(qi, 128)],
                            c_et[:, S - 128 - qi * 128 + c0:
                                 S - 128 - qi * 128 + c0 + cw],
                            start=True, stop=True)
                        nc.vector.tensor_copy(out=br[:, c0:c0 + cw],
                                              in_=pm[:, :cw])
                    # walrus rejects diagonal SBUF APs (partition-step
                    # semantics) — bounce through DRAM, where APs are linear.
                    brd = dram.tile([128, wp], BF, tag=f"brd{qi}",
                                    name=f"brd{h}_{qi}")
                    nc.gpsimd.dma_start(out=brd[:], in_=br[:])
                    brdap = brd[:]
                    # one 3D XBAR per q-tile: out[p, jb, f] = Sb[f, jb*128+p],
                    # alternating the two HWDGE queues (sync / scalar)
                    st3 = attn.tile([128, qi + 1, 128], BF, tag=f"sbT{qi}")
                    diag = bass.AP(tensor=brdap.tensor,
                                   offset=brdap.offset + 127,
                                   ap=[[wp - 1, 128], [1, w]])
                    eng = nc.sync if (h + qi) % 2 == 0 else nc.scalar
                    eng.dma_start_transpose(out=st3[:], in_=diag)
                    sbT.append(st3)

                # ---- AV + softmax-normalize + combine ----
                for qi in range(QT):
                    zp = ps_zp.tile([128, DA], F32, tag="zp")
                    n_mm = (QT - qi) + (QT if qi == QT - 1 else 0)
                    mi = 0
                    for lb in range(qi, QT):
                        nc.tensor.matmul(
                            zp[:], expT[lb][:, bass.ts(qi, 128)],
                            vN[:, lb, :],
                            start=(mi == 0), stop=(mi == n_mm - 1))
                        mi += 1
                    if qi == QT - 1:
                        for lb in range(QT):  # uniform last row fix
                            nc.tensor.matmul(
                                zp[:], c_e127[:], vN[:, lb, :],
                                start=(mi == 0), stop=(mi == n_mm - 1))
                            mi += 1
                    zs = ps_zs.tile([128, D], F32, tag="zs")
                    for jb in range(qi + 1):
                        nc.tensor.matmul(
                            zs[:], sbT[qi][:, jb, :], vN[:, jb, :D],
                            start=(jb == 0), stop=(jb == qi))
                    rcp = small.tile([128, 1], F32, tag="rcp")
                    nc.vector.reciprocal(rcp[:], zp[:, D:DA])
                    # DVE has a single PSUM port: zp and zs cannot be read in
                    # one op, so normalize then add in two steps.
                    zpn = small.tile([128, D], F32, tag="zpn")
                    nc.vector.tensor_scalar_mul(zpn[:], zp[:, :D], rcp[:])
                    zrow = small.tile([128, D], BF, tag="zrow")
                    nc.vector.tensor_add(zrow[:], zpn[:], zs[:])
                    # transpose z [128, 64] -> [64, 128] and place into zTc
                    zt = ps_zt.tile([D, 128], BF, tag="zt")
                    nc.tensor.transpose(zt[:], zrow[:], c_id[:])
                    nc.scalar.copy(
                        out=zTc[h // 2][(h % 2) * D:(h % 2) * D + D,
                                        bass.ts(qi, 128)],
                        in_=zt[:])

            # ---- output projection: outT[m-chunk] = WoT-chunks @ zTc + bo ----
            for m in range(4):
                for sl in range(2):
                    pm = ps_big.tile([128, 512], F32, tag="mm")
                    for kc in range(4):
                        nc.tensor.matmul(
                            pm[:], c_wo[:, kc, bass.ts(m, 128)],
                            zTc[kc][:, bass.ts(sl, 512)],
                            start=(kc == 0), stop=(kc == 3))
                    ot = outp.tile([128, 512], F32, tag="ot")
                    nc.scalar.add(out=ot[:], in_=pm[:], add=c_bo[:, m, :])
                    nc.gpsimd.dma_start(
                        out=outT[bass.ts(m, 128), bass.ts(sl, 512)],
                        in_=ot[:])

    nc.compile()
    return nc


_nc_cache = {}


def _get_nc():
    if "nc" not in _nc_cache:
        _nc_cache["nc"] = build_nc()
    return _nc_cache["nc"]


def _prep_inputs(v, k, q, Wv, bv, Wk, bk, Wq, bq, E, Wo, bo):
    """Host-side layout prep (numpy only). Returns per-core input maps."""
    f = np.asarray
    v, k, q = f(v, np.float32), f(k, np.float32), f(q, np.float32)
    Wv, bv = f(Wv, np.float32), f(bv, np.float32)
    Wk, bk = f(Wk, np.float32), f(bk, np.float32)
    Wq, bq = f(Wq, np.float32), f(bq, np.float32)
    E, Wo, bo = f(E, np.float32), f(Wo, np.float32), f(bo, np.float32)

    scale = 1.0 / np.sqrt(np.float32(EMB))

    def prep_x(x):  # (N,S,EMB) -> (N,H,DA,S) bf16
        xt = x.reshape(N, S, H, D).transpose(0, 2, 3, 1)
        ones = np.ones((N, H, 1, S), np.float32)
        return np.ascontiguousarray(
            np.concatenate([xt, ones], axis=2)).astype(bf16)

    vT_, kT_, qT_ = prep_x(v), prep_x(k), prep_x(q)

    def prep_w(W_, b_, s=1.0):
        return np.concatenate(
            [W_.T * s, b_[None, :] * s], 0).astype(bf16)

    wv, wk, wq = prep_w(Wv, bv), prep_w(Wk, bk), prep_w(Wq, bq, scale)
    wvn = np.zeros((DA, DA), np.float32)
    wvn[:D, :D] = Wv.T
    wvn[D, :D] = bv
    wvn[D, D] = 1.0
    wvn = wvn.astype(bf16)
    eta = np.ascontiguousarray(E[0].T).astype(bf16)          # [D, S]
    wot = np.ascontiguousarray(Wo.T).astype(bf16)            # [e, e_out]
    boc = np.ascontiguousarray(bo.reshape(4, 128, 1)).astype(np.float32)
    m01 = np.tril(np.ones((128, 128), np.float32), -1).astype(bf16)
    e127 = np.zeros((128, 128), np.float32)
    e127[:, 127] = 1.0 / S
    e127 = e127.astype(bf16)

    shared = {"WvTa": wv, "WkTa": wk, "WqTa": wq, "WvN": wvn, "ETa": eta,
              "WoT": wot, "boC": boc, "M01": m01, "E127": e127}
    return [
        {"vTa": np.ascontiguousarray(vT_[n]),
         "kTa": np.ascontiguousarray(kT_[n]),
         "qTa": np.ascontiguousarray(qT_[n]), **shared}
        for n in range(N)
    ]


def kernel(v, k, q, Wv, bv, Wk, bk, Wq, bq, E, Wo, bo):
    in_maps = _prep_inputs(v, k, q, Wv, bv, Wk, bk, Wq, bq, E, Wo, bo)
    nc = _get_nc()
    res = run_bass_kernel_spmd(nc, in_maps, list(range(NCORES)))
    out = np.stack([res.results[n]["outT"] for n in range(N)])  # (N,EMB,S)
    return np.ascontiguousarray(out.transpose(0, 2, 1))


# revision 9
# speedup vs baseline: 1.6149x; 1.0751x over previous
"""Trainium2 Bass kernel for nn_CrossAttention (sparse_attention).

Strategy: data-parallel over batch N=8 across the 8 NeuronCores; each core
computes one batch element's full attention independently (no collectives).

Per-core math (per head h, S=1024, D=64):
  vals/keys/qrs = per-head linear projections (bias folded in via an
    augmented ones-row; the 1/sqrt(EMB) softmax scale folded into Wq).
  scoresT[j,i] = keys[j]. qrs[i]          (computed transposed, strict j>i)
  P = exp(scoresT) with masked entries exactly 0 (multiply by 0/1 mask on
    diagonal blocks; lower blocks never computed). Softmax denominators come
    free from a ones-column appended to V in the AV matmul; the fully-masked
    last row (uniform 1/S) is reproduced exactly by a rank-1 constant matmul.
  Sbias[i,j] = QE-skew = vals[i] . E[S-1-i+j]  for j<=i (full lower tri).
    Implemented with ZERO reversals/transposes of B:
      B_rev[i-tile, c] = vals[i] . E[c + S-128-q0]   (plain slice of E^T)
      Sbias[i,j] = B_rev[i, 127 - (i%128) + j]       (diagonal SBUF AP)
    then XBAR dma_start_transpose per 128-block -> SbiasT for the matmul.
  z = (exp @ V)/Z + SbiasT-matmul, combined in one scalar_tensor_tensor.
  zT via PE transpose; out = Wo @ zT + bo (per 128-chunk), output transposed.
"""
import os

os.environ.setdefault("MYCRO_LOCAL_CACHE", "1")

import numpy as np
import ml_dtypes

import concourse.bass as bass
import concourse.mybir as mybir
import concourse.tile as tile
from concourse import bacc
from concourse.bass_utils import run_bass_kernel_spmd
from concourse.masks import make_identity

N, S, EMB, H, D = 8, 1024, 512, 8, 64
DA = D + 1              # augmented (ones row)
NCORES = 8
BF = mybir.dt.bfloat16
F32 = mybir.dt.float32
bf16 = ml_dtypes.bfloat16
QT = S // 128           # 8 q/l tiles per head


def build_nc(heads=H):
    nc = bacc.Bacc("TRN2", target_bir_lowering=False, debug=False,
                   num_devices=NCORES)

    # ---- DRAM I/O ----
    # augmented transposed inputs per head: [H, DA, S]
    vTa = nc.dram_tensor("vTa", [H, DA, S], BF, kind="ExternalInput")
    kTa = nc.dram_tensor("kTa", [H, DA, S], BF, kind="ExternalInput")
    qTa = nc.dram_tensor("qTa", [H, DA, S], BF, kind="ExternalInput")
    WvTa = nc.dram_tensor("WvTa", [DA, D], BF, kind="ExternalInput")
    WkTa = nc.dram_tensor("WkTa", [DA, D], BF, kind="ExternalInput")
    WqTa = nc.dram_tensor("WqTa", [DA, D], BF, kind="ExternalInput")
    WvN = nc.dram_tensor("WvN", [DA, DA], BF, kind="ExternalInput")
    ETa = nc.dram_tensor("ETa", [D, S], BF, kind="ExternalInput")
    WoT = nc.dram_tensor("WoT", [EMB, EMB], BF, kind="ExternalInput")
    boC = nc.dram_tensor("boC", [4, 128, 1], F32, kind="ExternalInput")
    M01 = nc.dram_tensor("M01", [128, 128], BF, kind="ExternalInput")
    E127 = nc.dram_tensor("E127", [128, 128], BF, kind="ExternalInput")
    outT = nc.dram_tensor("outT", [EMB, S], F32, kind="ExternalOutput")

    from contextlib import ExitStack
    with tile.TileContext(nc) as tc, ExitStack() as ctx:
        consts = ctx.enter_context(tc.tile_pool(name="consts", bufs=1))
        inp = ctx.enter_context(tc.tile_pool(name="inp", bufs=2))
        proj = ctx.enter_context(tc.tile_pool(name="proj", bufs=2))
        attn = ctx.enter_context(tc.tile_pool(name="attn", bufs=3))
        small = ctx.enter_context(tc.tile_pool(name="small", bufs=4))
        zt_pool = ctx.enter_context(tc.tile_pool(name="ztp", bufs=1))
        outp = ctx.enter_context(tc.tile_pool(name="outp", bufs=2))
        # PSUM pools (8 banks total): big rotating pool + small dedicated
        ps_big = ctx.enter_context(tc.tile_pool(name="ps_big", bufs=3,
                                                space="PSUM"))
        ps_zp = ctx.enter_context(tc.tile_pool(name="ps_zp", bufs=2,
                                               space="PSUM"))
        ps_zs = ctx.enter_context(tc.tile_pool(name="ps_zs", bufs=2,
                                               space="PSUM"))
        ps_zt = ctx.enter_context(tc.tile_pool(name="ps_zt", bufs=1,
                                               space="PSUM"))
        dram = ctx.enter_context(tc.tile_pool(name="dram", bufs=2,
                                              space="DRAM"))
        if True:
            # ---- load constants ----
            c_wv = consts.tile([DA, D], BF)
            c_wk = consts.tile([DA, D], BF)
            c_wq = consts.tile([DA, D], BF)
            c_wvn = consts.tile([DA, DA], BF)
            c_et = consts.tile([D, S], BF)
            c_wo = consts.tile([128, 4, EMB], BF)   # [e-part, e-chunk, e_out]
            c_bo = consts.tile([128, 4, 1], F32)
            c_m01 = consts.tile([128, 128], BF)
            c_e127 = consts.tile([128, 128], BF)
            c_id = consts.tile([128, 128], BF)
            nc.sync.dma_start(out=c_wv[:], in_=WvTa[:])
            nc.sync.dma_start(out=c_wk[:], in_=WkTa[:])
            nc.sync.dma_start(out=c_wq[:], in_=WqTa[:])
            nc.sync.dma_start(out=c_wvn[:], in_=WvN[:])
            nc.sync.dma_start(out=c_et[:], in_=ETa[:])
            nc.sync.dma_start(
                out=c_wo[:], in_=WoT[:].rearrange("(c p) e -> p c e", p=128))
            nc.sync.dma_start(
                out=c_bo[:], in_=boC[:].rearrange("c p one -> p c one"))
            nc.sync.dma_start(out=c_m01[:], in_=M01[:])
            nc.sync.dma_start(out=c_e127[:], in_=E127[:])
            make_identity(nc, c_id[:])

            # zT chunks: [128 (= 2 heads of d), S] bf16, 4 of them
            zTc = [zt_pool.tile([128, S], BF, tag=f"ztc{i}", name=f"ztc{i}")
                   for i in range(4)]

            for h in range(heads):
                # ---- stage inputs ----
                xv = inp.tile([DA, S], BF, tag="xv")
                xk = inp.tile([DA, S], BF, tag="xk")
                xq = inp.tile([DA, S], BF, tag="xq")
                nc.gpsimd.dma_start(out=xv[:], in_=vTa[h])
                nc.gpsimd.dma_start(out=xk[:], in_=kTa[h])
                nc.gpsimd.dma_start(out=xq[:], in_=qTa[h])

                # ---- transposed projections kT/qT/vT [64, S] ----
                kT = proj.tile([D, S], BF, tag="kT")
                qT = proj.tile([D, S], BF, tag="qT")
                vT = proj.tile([D, S], BF, tag="vT")
                for (dst, w, x) in ((kT, c_wk, xk), (qT, c_wq, xq),
                                    (vT, c_wv, xv)):
                    for sl in range(2):
                        cols = bass.ts(sl, 512)
                        pm = ps_big.tile([128, 512], F32, tag="mm")
                        nc.tensor.matmul(pm[:D, :], w[:], x[:, cols],
                                         start=True, stop=True)
                        nc.scalar.copy(out=dst[:, cols], in_=pm[:D, :])

                # ---- normal-form vals with ones column: vN [128, QT, DA] ----
                vN = proj.tile([128, QT, DA], BF, tag="vN")
                for g in range(2):  # two psum groups of 4 l-tiles
                    pm = ps_big.tile([128, 4 * DA], F32, tag="mm")
                    for lt4 in range(4):
                        lt = g * 4 + lt4
                        nc.tensor.matmul(
                            pm[:, lt4 * DA:(lt4 + 1) * DA],
                            xv[:, bass.ts(lt, 128)], c_wvn[:],
                            start=True, stop=True)
                    nc.vector.tensor_copy(
                        out=vN[:, g * 4:(g + 1) * 4, :].rearrange(
                            "p a b -> p (a b)"),
                        in_=pm[:])

                # ---- scoresT -> exp (per l-tile lb), strict upper j>i ----
                # expT[lb]: [128, 128*(lb+1)] bf16
                expT = []
                for lb in range(QT):
                    wl = 128 * (lb + 1)
                    et = attn.tile([128, wl], BF, tag=f"expT{lb}")
                    for c0 in range(0, wl, 512):
                        cw = min(512, wl - c0)
                        pm = ps_big.tile([128, 512], F32, tag="mm")
                        nc.tensor.matmul(pm[:, :cw],
                                         kT[:, bass.ts(lb, 128)],
                                         qT[:, c0:c0 + cw],
                                         start=True, stop=True)
                        nc.scalar.activation(
                            out=et[:, c0:c0 + cw], in_=pm[:, :cw],
                            func=mybir.ActivationFunctionType.Exp)
                    # diagonal block: keep only j>i  (p>f within block)
                    nc.vector.tensor_mul(et[:, bass.ts(lb, 128)],
                                         et[:, bass.ts(lb, 128)], c_m01[:])
                    expT.append(et)

                # ---- B_rev + skew-transpose -> SbT blocks ----
                # B_rev[qi]: [128, 128*(qi+1) + 128] bf16 (tail 128 zeroed)
                sbT = []
                for qi in range(QT):
                    w = 128 * (qi + 1)
                    wp = w + 128
                    br = attn.tile([128, wp], BF, tag=f"brev{qi}")
                    nc.vector.memset(br[:, w:wp], 0.0)
                    for c0 in range(0, w, 512):
                        cw = min(512, w - c0)
                        pm = ps_big.tile([128, 512], F32, tag="mm")
                        nc.tensor.matmul(
                            pm[:, :cw],
                            vT[:, bass.ts(qi, 128)],
                            c_et[:, S - 128 - qi * 128 + c0:
                                 S - 128 - qi * 128 + c0 + cw],
                            start=True, stop=True)
                        nc.vector.tensor_copy(out=br[:, c0:c0 + cw],
                                              in_=pm[:, :cw])
                    # walrus rejects diagonal SBUF APs (partition-step
                    # semantics) — bounce through DRAM, where APs are linear.
                    brd = dram.tile([128, wp], BF, tag=f"brd{qi}",
                                    name=f"brd{h}_{qi}")
                    nc.gpsimd.dma_start(out=brd[:], in_=br[:])
                    brdap = brd[:]
                    # one 3D XBAR per q-tile: out[p, jb, f] = Sb[f, jb*128+p],
                    # alternating the two HWDGE queues (sync / scalar)
                    st3 = attn.tile([128, qi + 1, 128], BF, tag=f"sbT{qi}")
                    diag = bass.AP(tensor=brdap.tensor,
                                   offset=brdap.offset + 127,
                                   ap=[[wp - 1, 128], [1, w]])
                    eng = nc.sync if (h + qi) % 2 == 0 else nc.scalar
                    eng.dma_start_transpose(out=st3[:], in_=diag)
                    sbT.append(st3)

                # ---- AV + softmax-normalize + combine ----
                for qi in range(QT):
                    zp = ps_zp.tile([128, DA], F32, tag="zp")
                    n_mm = (QT - qi) + (QT if qi == QT - 1 else 0)
                    mi = 0
                    for lb in range(qi, QT):
                        nc.tensor.matmul(
                            zp[:], expT[lb][:, bass.ts(qi, 128)],
                            vN[:, lb, :],
                            start=(mi == 0), stop=(mi == n_mm - 1))
                        mi += 1
                    if qi == QT - 1:
                        for lb in range(QT):  # uniform last row fix
                            nc.tensor.matmul(
                                zp[:], c_e127[:], vN[:, lb, :],
                                start=(mi == 0), stop=(mi == n_mm - 1))
                            mi += 1
                    zs = ps_zs.tile([128, D], F32, tag="zs")
                    for jb in range(qi + 1):
                        nc.tensor.matmul(
                            zs[:], sbT[qi][:, jb, :], vN[:, jb, :D],
                            start=(jb == 0), stop=(jb == qi))
                    rcp = small.tile([128, 1], F32, tag="rcp")
                    nc.vector.reciprocal(rcp[:], zp[:, D:DA])
                    # DVE has a single PSUM port: zp and zs cannot be read in
                    # one op, so normalize then add in two steps.
                    zpn = small.tile([128, D], F32, tag="zpn")
                    nc.vector.tensor_scalar_mul(zpn[:], zp[:, :D], rcp[:])
                    zrow = small.tile([128, D], BF, tag="zrow")
                    nc.vector.tensor_add(zrow[:], zpn[:], zs[:])
                    # transpose z [128, 64] -> [64, 128] and place into zTc
                    zt = ps_zt.tile([D, 128], BF, tag="zt")
                    nc.tensor.transpose(zt[:], zrow[:], c_id[:])
                    nc.scalar.copy(
                        out=zTc[h // 2][(h % 2) * D:(h % 2) * D + D,
                                        bass.ts(qi, 128)],
                        in_=zt[:])

            # ---- output projection: outT[m-chunk] = WoT-chunks @ zTc + bo ----
            for m in range(4):
                for sl in range(2):
                    pm = ps_big.tile([128, 512], F32, tag="mm")
                    for kc in range(4):
                        nc.tensor.matmul(
                            pm[:], c_wo[:, kc, bass.ts(m, 128)],
                            zTc[kc][:, bass.ts(sl, 512)],
                            start=(kc == 0), stop=(kc == 3))
                    ot = outp.tile([128, 512], F32, tag="ot")
                    nc.scalar.add(out=ot[:], in_=pm[:], add=c_bo[:, m, :])
                    nc.gpsimd.dma_start(
                        out=outT[bass.ts(m, 128), bass.ts(sl, 512)],
                        in_=ot[:])

    nc.compile()
    return nc


_nc_cache = {}


def _get_nc():
    if "nc" not in _nc_cache:
        _nc_cache["nc"] = build_nc()
    return _nc_cache["nc"]


def _prep_inputs(v, k, q, Wv, bv, Wk, bk, Wq, bq, E, Wo, bo):
    """Host-side layout prep (numpy only). Returns per-core input maps."""
    f = np.asarray
    v, k, q = f(v, np.float32), f(k, np.float32), f(q, np.float32)
    Wv, bv = f(Wv, np.float32), f(bv, np.float32)
    Wk, bk = f(Wk, np.float32), f(bk, np.float32)
    Wq, bq = f(Wq, np.float32), f(bq, np.float32)
    E, Wo, bo = f(E, np.float32), f(Wo, np.float32), f(bo, np.float32)

    scale = 1.0 / np.sqrt(np.float32(EMB))

    def prep_x(x):  # (N,S,EMB) -> (N,H,DA,S) bf16
        xt = x.reshape(N, S, H, D).transpose(0, 2, 3, 1)
        ones = np.ones((N, H, 1, S), np.float32)
        return np.ascontiguousarray(
            np.concatenate([xt, ones], axis=2)).astype(bf16)

    vT_, kT_, qT_ = prep_x(v), prep_x(k), prep_x(q)

    def prep_w(W_, b_, s=1.0):
        return np.concatenate(
            [W_.T * s, b_[None, :] * s], 0).astype(bf16)

    wv, wk, wq = prep_w(Wv, bv), prep_w(Wk, bk), prep_w(Wq, bq, scale)
    wvn = np.zeros((DA, DA), np.float32)
    wvn[:D, :D] = Wv.T
    wvn[D, :D] = bv
    wvn[D, D] = 1.0
    wvn = wvn.astype(bf16)
    eta = np.ascontiguousarray(E[0].T).astype(bf16)          # [D, S]
    wot = np.ascontiguousarray(Wo.T).astype(bf16)            # [e, e_out]
    boc = np.ascontiguousarray(bo.reshape(4, 128, 1)).astype(np.float32)
    m01 = np.tril(np.ones((128, 128), np.float32), -1).astype(bf16)
    e127 = np.zeros((128, 128), np.float32)
    e127[:, 127] = 1.0 / S
    e127 = e127.astype(bf16)

    shared = {"WvTa": wv, "WkTa": wk, "WqTa": wq, "WvN": wvn, "ETa": eta,
              "WoT": wot, "boC": boc, "M01": m01, "E127": e127}
    return [
        {"vTa": np.ascontiguousarray(vT_[n]),
         "kTa": np.ascontiguousarray(kT_[n]),
         "qTa": np.ascontiguousarray(qT_[n]), **shared}
        for n in range(N)
    ]


def kernel(v, k, q, Wv, bv, Wk, bk, Wq, bq, E, Wo, bo):
    in_maps = _prep_inputs(v, k, q, Wv, bv, Wk, bk, Wq, bq, E, Wo, bo)
    nc = _get_nc()
    res = run_bass_kernel_spmd(nc, in_maps, list(range(NCORES)))
    out = np.stack([res.results[n]["outT"] for n in range(N)])  # (N,EMB,S)
    return np.ascontiguousarray(out.transpose(0, 2, 1))
